# revision 1
# baseline (speedup 1.0000x reference)
"""Trainium2 Bass kernel for nn_Coords2RMSD (masked Kabsch RMSD loss).

Pure data parallel over 8 NeuronCores (1024 samples each). Inputs are
pre-planarized on the host (each row [x1(768)|x2(768)|x3(768)]) so every
device op is contiguous. Per core, samples are processed in 8 tiles of
128 (partition = sample). Each tile's X/Y rows stream from HBM once; 17
per-sample reductions (mask-weighted component sums, sums of squares, and
the 3x3 correlation matrix) are computed with fused multiply-accumulate
ops spread across DVE / GPSIMD / ACT. A closed-form 3x3 eigenvalue
epilogue (trig method; polynomial acos/sin/cos; sqrt via exp(0.5*ln))
turns the reductions into the RMSD.
"""
import math
import numpy as np

P = 128          # partitions (samples per tile)
M = 768          # max atoms
D = 3 * M        # row length
NCORES = 8
T = 8            # tiles per core
S = P * T        # samples per core
YM_DVE = 1536    # elements of ym built on DVE (rest on gpsimd)

_CACHE = {}


def _build(n_tiles):
    import concourse.bacc as bacc
    import concourse.mybir as mybir
    from concourse.tile import TileContext
    from concourse.hw_specs import get_activation_tables

    f32 = mybir.dt.float32
    bf16 = mybir.dt.bfloat16
    ALU = mybir.AluOpType
    AF = mybir.ActivationFunctionType

    Tn = n_tiles
    Sn = P * Tn

    nc = bacc.Bacc()
    xd = nc.declare_dram_parameter("x", [Sn, D], f32, isOutput=False)
    yd = nc.declare_dram_parameter("y", [Sn, D], f32, isOutput=False)
    # consts packs [iota_planar (D) | nv (Tn) | invn (Tn)]
    constsd = nc.declare_dram_parameter("consts", [P, D + 2 * Tn], f32,
                                        isOutput=False)
    outd = nc.declare_dram_parameter("out", [P, Tn], f32, isOutput=True)

    with TileContext(nc) as tc:
        with tc.tile_pool(name="io", bufs=3) as io, \
             tc.tile_pool(name="wk", bufs=2) as wk, \
             tc.tile_pool(name="st", bufs=1) as st:
            consts_t = st.tile([P, D + 2 * Tn], f32)
            nc.sync.dma_start(out=consts_t[:], in_=constsd[:])
            iota_t = consts_t[:, 0:D]           # planar atom index (x3)
            nv_t = consts_t[:, D:D + Tn]
            invn_t = consts_t[:, D + Tn:D + 2 * Tn]

            # stats accumulators
            mm = st.tile([P, 9 * Tn], f32)    # col (i*3+j)*Tn + t
            sx = st.tile([P, 3 * Tn], f32)    # col i*Tn + t
            sy = st.tile([P, 3 * Tn], f32)
            ssx = st.tile([P, Tn], f32)
            ssy = st.tile([P, Tn], f32)

            for t in range(Tn):
                xt = io.tile([P, D], f32, tag="x")
                nc.sync.dma_start(out=xt[:], in_=xd[t * P:(t + 1) * P, :])
                yt = io.tile([P, D], f32, tag="y")
                nc.sync.dma_start(out=yt[:], in_=yd[t * P:(t + 1) * P, :])

                # mask = (iota < n), planar, on DVE (single-src 2x mode)
                mask3 = wk.tile([P, D], f32, tag="mask3")
                nc.vector.tensor_scalar(out=mask3[:], in0=iota_t,
                                        scalar1=nv_t[:, t:t + 1], scalar2=None,
                                        op0=ALU.is_lt)

                # masked tensors in bf16 (cast on write): xm + ym head on
                # gpsimd, ym tail on DVE
                xm = wk.tile([P, D], bf16, tag="xm")
                nc.gpsimd.tensor_tensor(out=xm[:], in0=xt[:], in1=mask3[:],
                                        op=ALU.mult)
                ym = wk.tile([P, D], bf16, tag="ym")
                h = D - YM_DVE
                nc.gpsimd.tensor_tensor(out=ym[:, :h], in0=yt[:, :h],
                                        in1=mask3[:, :h], op=ALU.mult)
                nc.vector.tensor_tensor(out=ym[:, h:], in0=yt[:, h:],
                                        in1=mask3[:, h:], op=ALU.mult)

                # products: m_ij = sum_a xm_i * ym_j (fused accumulate, DVE,
                # bf16 inputs for the 2x perf mode; fp32 accumulator)
                for i in range(3):
                    for j in range(3):
                        junk = wk.tile([P, M], bf16, tag="junk")
                        col = (i * 3 + j) * Tn + t
                        nc.vector.scalar_tensor_tensor(
                            out=junk[:], in0=xm[:, i * M:(i + 1) * M],
                            scalar=1.0, in1=ym[:, j * M:(j + 1) * M],
                            op0=ALU.mult, op1=ALU.mult,
                            accum_out=mm[:, col:col + 1])

                # X sumsq on DVE (bf16 2x), Y sumsq on ACT, comp sums on ACT
                junk2 = wk.tile([P, D], bf16, tag="junk2")
                nc.vector.scalar_tensor_tensor(
                    out=junk2[:], in0=xm[:], scalar=1.0, in1=xm[:],
                    op0=ALU.mult, op1=ALU.mult,
                    accum_out=ssx[:, t:t + 1])
                sq2 = wk.tile([P, D], bf16, tag="sq")
                nc.scalar.activation(out=sq2[:], in_=ym[:], func=AF.Square,
                                     accum_out=ssy[:, t:t + 1])
                for i in range(3):
                    cp2 = wk.tile([P, M], bf16, tag="cp")
                    nc.scalar.activation(out=cp2[:], in_=ym[:, i * M:(i + 1) * M],
                                         func=AF.Copy,
                                         accum_out=sy[:, i * Tn + t:i * Tn + t + 1])
                    cp = wk.tile([P, M], bf16, tag="cp")
                    nc.scalar.activation(out=cp[:], in_=xm[:, i * M:(i + 1) * M],
                                         func=AF.Copy,
                                         accum_out=sx[:, i * Tn + t:i * Tn + t + 1])

            # ---------------- epilogue (batched over [P, ..., Tn]) ----------
            cnt = [0]

            def new(shape):
                """Allocate a scratch tile; return an AP shaped like `shape`."""
                cnt[0] += 1
                free = int(np.prod(shape[1:]))
                r = st.tile([P, free], f32, tag=f"e{cnt[0]}")
                ap = r[:]
                if len(shape) > 2:
                    names = " ".join(f"d{i}" for i in range(len(shape) - 1))
                    ap = ap.rearrange(f"p ({names}) -> p {names}",
                                      **{f"d{i}": int(shape[1 + i])
                                         for i in range(len(shape) - 1)})
                return ap

            def tt(a, b, op, shape=None):
                r = new(list(shape or a.shape))
                nc.vector.tensor_tensor(out=r, in0=a, in1=b, op=op)
                return r

            def ts(a, s1, op0, s2=None, op1=None):
                r = new(list(a.shape))
                if op1 is None:
                    nc.vector.tensor_scalar(out=r, in0=a, scalar1=s1,
                                            scalar2=None, op0=op0)
                else:
                    nc.vector.tensor_scalar(out=r, in0=a, scalar1=s1,
                                            scalar2=s2, op0=op0, op1=op1)
                return r

            def stt(a, s, b, op0, op1):
                r = new(list(a.shape))
                nc.vector.scalar_tensor_tensor(out=r, in0=a, scalar=s,
                                               in1=b, op0=op0, op1=op1)
                return r

            def act(a, func, scale=1.0, bias=0.0):
                r = new(list(a.shape))
                nc.scalar.activation(out=r, in_=a, func=func,
                                     scale=scale, bias=bias)
                return r

            def recip(a):
                r = new(list(a.shape))
                nc.vector.reciprocal(out=r, in_=a)
                return r

            def red_inner(a, n_keep):
                r = new([P, n_keep])
                nc.vector.tensor_reduce(out=r, in_=a,
                                        axis=mybir.AxisListType.X, op=ALU.add)
                return r

            def poly_eval(x, coeffs):
                """coeffs [a_n..a_1, a_0] -> a_0 + x*(a_1 + x*(...a_n))"""
                g = ts(x, coeffs[0], ALU.mult)
                for c in coeffs[1:-1]:
                    g = stt(g, c, x, ALU.add, ALU.mult)
                return ts(g, coeffs[-1], ALU.add)

            mmv = mm[:].rearrange("p (i j t) -> p i j t", i=3, j=3)
            sxv = sx[:].rearrange("p (i t) -> p i t", i=3)
            syv = sy[:].rearrange("p (i t) -> p i t", i=3)
            invn_b3 = invn_t.unsqueeze(1).broadcast_to([P, 3, Tn])

            # R_ij = m_ij - (sx_i * invn) * sy_j
            meanx = tt(sxv, invn_b3, ALU.mult)                       # [P,3,Tn]
            meanx_v = meanx.unsqueeze(2).broadcast_to([P, 3, 3, Tn])
            sy_v = syv.unsqueeze(1).broadcast_to([P, 3, 3, Tn])
            mxsy = tt(meanx_v, sy_v, ALU.mult)
            Rv = tt(mmv, mxsy, ALU.subtract)                         # [P,3,3,Tn]

            # e0 = ssx + ssy - (|sx|^2 + |sy|^2) * invn
            sx2 = tt(sxv, sxv, ALU.mult)
            sy2 = tt(syv, syv, ALU.mult)
            nrm = tt(sx2, sy2, ALU.add)
            nrms = red_inner(nrm.rearrange("p i t -> p t i"), Tn)
            ss = tt(ssx[:], ssy[:], ALU.add)
            nrmi = tt(nrms, invn_t, ALU.mult)
            e0 = tt(ss, nrmi, ALU.subtract)                          # [P,Tn]

            # A = R^T R (batched outer products over k)
            Av = new([P, 3, 3, Tn])
            for k in range(3):
                rk = Rv[:, k]
                rk_a = rk.unsqueeze(2).broadcast_to([P, 3, 3, Tn])
                rk_b = rk.unsqueeze(1).broadcast_to([P, 3, 3, Tn])
                if k == 0:
                    nc.vector.tensor_tensor(out=Av, in0=rk_a, in1=rk_b,
                                            op=ALU.mult)
                else:
                    pk = tt(rk_a, rk_b, ALU.mult)
                    nc.vector.tensor_tensor(out=Av, in0=Av, in1=pk, op=ALU.add)
            Aflat = Av.rearrange("p a b t -> p (a b) t")
            Adiag = Aflat[:, ::4]                                    # [P,3,Tn]

            q = ts(red_inner(Adiag.rearrange("p a t -> p t a"), Tn),
                   1.0 / 3.0, ALU.mult)                              # [P,Tn]
            q_b3 = q.unsqueeze(1).broadcast_to([P, 3, Tn])
            bdiag = tt(Adiag, q_b3, ALU.subtract)

            # p2 = sum(bdiag^2) + (sum(A^2) - sum(diag(A)^2))
            asq = tt(Aflat, Aflat, ALU.mult)
            allsq = red_inner(asq.rearrange("p a t -> p t a"), Tn)
            dsq = tt(Adiag, Adiag, ALU.mult)
            dsqs = red_inner(dsq.rearrange("p a t -> p t a"), Tn)
            bsq = tt(bdiag, bdiag, ALU.mult)
            bsqs = red_inner(bsq.rearrange("p a t -> p t a"), Tn)
            offs = tt(allsq, dsqs, ALU.subtract)
            p2 = tt(bsqs, offs, ALU.add)                             # [P,Tn]

            # log-space: p = (p2/6)^0.5 and invp^3 = (p2/6)^-1.5
            p2e = ts(p2, 1e-10, ALU.add)
            lnp2 = act(p2e, AF.Ln, scale=1.0 / 6.0)
            p_ = act(lnp2, AF.Exp, scale=0.5)
            ip3 = act(lnp2, AF.Exp, scale=-1.5)

            # batched determinants of W0=R and W1=B (= A - q I)
            Dw = new([P, 2, 3, 3, Tn])
            nc.vector.tensor_copy(Dw[:, 0], Rv)
            nc.vector.tensor_copy(Dw[:, 1], Av)
            Dw_diag = Dw.rearrange("p w a b t -> p w (a b) t")[:, 1, ::4]
            nc.vector.tensor_tensor(out=Dw_diag, in0=Adiag, in1=q_b3,
                                    op=ALU.subtract)

            def dsl(i, j):
                return Dw[:, :, i, j]                                # [P,2,Tn]

            u1 = tt(dsl(1, 1), dsl(2, 2), ALU.mult)
            u2 = tt(dsl(1, 2), dsl(2, 1), ALU.mult)
            cof0 = tt(dsl(0, 0), tt(u1, u2, ALU.subtract), ALU.mult)
            u3 = tt(dsl(1, 0), dsl(2, 2), ALU.mult)
            u4 = tt(dsl(1, 2), dsl(2, 0), ALU.mult)
            cof1 = tt(dsl(0, 1), tt(u3, u4, ALU.subtract), ALU.mult)
            u5 = tt(dsl(1, 0), dsl(2, 1), ALU.mult)
            u6 = tt(dsl(1, 1), dsl(2, 0), ALU.mult)
            cof2 = tt(dsl(0, 2), tt(u5, u6, ALU.subtract), ALU.mult)
            dets = tt(tt(cof0, cof1, ALU.subtract), cof2, ALU.add)   # [P,2,Tn]
            detR = dets[:, 0]
            detB = dets[:, 1]

            # r = clamp(0.5 * detB * invp^3, -1, 1)
            rr = tt(detB, ip3, ALU.mult)
            r_ = ts(rr, 0.5, ALU.mult, 1.0, ALU.min)
            r_ = ts(r_, -1.0, ALU.max)

            # acos(r)/3 via |r| polynomial (A&S 4.4.46) + reflection
            rneg = ts(r_, -1.0, ALU.mult)
            tabs = tt(r_, rneg, ALU.max)
            poly = poly_eval(tabs, [-0.0012624911, 0.0066700901, -0.0170881256,
                                    0.0308918810, -0.0501743046, 0.0889789874,
                                    -0.2145988016, 1.5707963050])
            u_ = ts(tabs, -1.0, ALU.mult, 1.0, ALU.add)
            u_ = ts(u_, 1e-30, ALU.add)
            sq1mt = act(act(u_, AF.Ln), AF.Exp, scale=0.5)
            acos_t = tt(poly, sq1mt, ALU.mult)
            ind = ts(r_, 0.0, ALU.is_ge)
            sgn = ts(ind, 2.0, ALU.mult, -1.0, ALU.add)
            pio = ts(ind, -math.pi, ALU.mult, math.pi, ALU.add)
            acos_r = tt(tt(acos_t, sgn, ALU.mult), pio, ALU.add)
            phi = ts(acos_r, 1.0 / 3.0, ALU.mult)

            # cos/sin Taylor on [0, pi/3]; cos(phi+2pi/3) = -.5 c - (v3/2) s
            z = tt(phi, phi, ALU.mult)
            cosp = poly_eval(z, [1.0 / 40320, -1.0 / 720, 1.0 / 24, -0.5, 1.0])
            sinp = poly_eval(z, [-1.0 / 5040, 1.0 / 120, -1.0 / 6, 1.0])
            sinp = tt(sinp, phi, ALU.mult)
            halfc = ts(cosp, -0.5, ALU.mult)
            cosp2 = stt(sinp, -math.sqrt(3.0) / 2.0, halfc, ALU.mult, ALU.add)

            twop = ts(p_, 2.0, ALU.mult)
            eigs = new([P, 3, Tn])
            e1t = tt(twop, cosp, ALU.mult)
            nc.vector.tensor_tensor(out=eigs[:, 0], in0=e1t, in1=q, op=ALU.add)
            e3t = tt(twop, cosp2, ALU.mult)
            nc.vector.tensor_tensor(out=eigs[:, 2], in0=e3t, in1=q, op=ALU.add)
            q3 = ts(q, 3.0, ALU.mult)
            e12 = tt(eigs[:, 0], eigs[:, 2], ALU.add)
            nc.vector.tensor_tensor(out=eigs[:, 1], in0=q3, in1=e12,
                                    op=ALU.subtract)

            eig_c = ts(eigs.rearrange("p k t -> p (k t)"), 0.0, ALU.max,
                       1e-30, ALU.add)                                # [P,3Tn]
            sv = act(act(eig_c, AF.Ln), AF.Exp, scale=0.5)
            sv = sv.rearrange("p (k t) -> p k t", k=3)

            dind = ts(detR, 0.0, ALU.is_ge)
            dsgn = ts(dind, 2.0, ALU.mult, -1.0, ALU.add)
            s12 = tt(sv[:, 0], sv[:, 1], ALU.add)
            ds3 = tt(dsgn, sv[:, 2], ALU.mult)
            trace = tt(s12, ds3, ALU.add)                             # [P,Tn]

            e_ = stt(trace, -2.0, e0, ALU.mult, ALU.add)
            e_ = ts(e_, 0.0, ALU.max)
            arg = tt(e_, invn_t, ALU.mult)
            arg = ts(arg, 1e-7, ALU.add)
            y0 = act(act(arg, AF.Ln), AF.Exp, scale=0.5)
            ry = recip(y0)
            ay = tt(arg, ry, ALU.mult)
            outv = ts(tt(y0, ay, ALU.add), 0.5, ALU.mult)

            nc.sync.dma_start(out=outd[:], in_=outv)

    nc.compile()

    # collapse redundant ACT table loads: every function we use (Copy,
    # Square, Ln, Exp) lives in natural_log_exp_and_others, but the
    # chooser ping-pongs between smaller sets. Retarget all loads to the
    # combined set and drop the now-redundant ones (keeping any that
    # carry sync commands).
    tables = list(get_activation_tables(nc.m.arch).keys())
    target = tables.index("natural_log_exp_and_others")
    for blk in nc.main_func.blocks:
        seen = False
        drop = []
        for inst in list(blk.instructions):
            if isinstance(inst, mybir.InstLoadActFuncSet):
                inst.act_func_set_id = target
                si = inst.sync_info
                has_sync = si is not None and (si.on_wait or si.on_update)
                if seen and not has_sync:
                    drop.append(inst)
                    continue
                seen = True
        for inst in drop:
            blk.instructions.remove(inst)
    return nc


def get_nc(n_tiles=T):
    if n_tiles not in _CACHE:
        _CACHE[n_tiles] = _build(n_tiles)
    return _CACHE[n_tiles]


def _planarize(A):
    """[B, (a c)] -> [B, (c a)] rows."""
    B = A.shape[0]
    return np.ascontiguousarray(
        A.reshape(B, M, 3).transpose(0, 2, 1).reshape(B, D))


def _prep_core_inputs(X, Y, nf, n_tiles):
    invn = (np.float32(1.0) / nf).astype(np.float32)
    consts = np.empty((P, D + 2 * n_tiles), np.float32)
    consts[:, 0:D] = np.tile(np.arange(M, dtype=np.float32), 3)[None, :]
    consts[:, D:D + n_tiles] = nf.reshape(n_tiles, P).T
    consts[:, D + n_tiles:] = invn.reshape(n_tiles, P).T
    return {
        "x": _planarize(X),
        "y": _planarize(Y),
        "consts": consts,
    }


def kernel(input, target, num_atoms):
    from concourse.bass_utils import run_bass_kernel_spmd

    X = np.asarray(input, dtype=np.float32)
    Y = np.asarray(target, dtype=np.float32)
    nf = np.asarray(num_atoms).astype(np.float32)
    B = X.shape[0]
    assert B == NCORES * S, f"unexpected batch {B}"

    nc = get_nc(T)
    in_maps = []
    for c in range(NCORES):
        sl = slice(c * S, (c + 1) * S)
        in_maps.append(_prep_core_inputs(X[sl], Y[sl], nf[sl], T))
    res = run_bass_kernel_spmd(nc, in_maps, list(range(NCORES))).results
    out = np.empty((NCORES, S), np.float32)
    for c in range(NCORES):
        out[c] = res[c]["out"].T.reshape(S)   # out[p,t] -> sample t*P+p
    return out.reshape(B)



# revision 2
# speedup vs baseline: 1.1468x; 1.1468x over previous
"""Trainium2 Bass kernel for nn_Coords2RMSD (masked Kabsch RMSD loss).

Pure data parallel over 8 NeuronCores (1024 samples each). All 17
per-sample reductions (3x3 correlation, component sums, sums of squares,
atom count) are computed on the TensorEngine as a batched 7x7 Gram
matrix: per group of 16 samples, Z = [X1 X2 X3 Y1 Y2 Y3 one] columns
(7 per sample, 112 cols) contract over atoms in 6 chunks of 128
partitions, accumulating G = Z^T (mask * Z) in PSUM. Masking only the
moving operand is exact (the mask is idempotent). Atoms < 384 are never
masked (num_atoms >= 384), so only chunks 3-5 need the mask multiply.
The per-sample 7x7 diagonal blocks are gathered sample-major via a
through-DRAM DMA shuffle, then a closed-form 3x3 eigenvalue epilogue
(trig method) turns the reductions into the RMSD.
"""
import math
import numpy as np

P = 128          # partitions
M = 768          # max atoms
NCORES = 8
T = 8            # epilogue tiles (samples p of tile t is sample 128t+p)
S = 1024         # samples per core
G = 64           # sample groups per core (16 samples each)
W = 112          # matmul columns per group (16 samples x 7 comps)
CH = 6           # contraction chunks of 128 atoms

_CACHE = {}


def _build(n_tiles):
    import concourse.bacc as bacc
    import concourse.mybir as mybir
    from concourse.tile import TileContext
    from concourse.hw_specs import get_activation_tables

    f32 = mybir.dt.float32
    bf16 = mybir.dt.bfloat16
    ALU = mybir.AluOpType
    AF = mybir.ActivationFunctionType

    Tn = T

    nc = bacc.Bacc()
    xzd = nc.declare_dram_parameter("xz", [G, P, CH * W], bf16, isOutput=False)
    mskd = nc.declare_dram_parameter("msk", [G, P, 48], bf16, isOutput=False)
    constsd = nc.declare_dram_parameter("consts", [P, 2 * Tn], f32,
                                        isOutput=False)
    outd = nc.declare_dram_parameter("out", [P, Tn], f32, isOutput=True)
    # scratch for the diagonal-block gather: [s(16), g(64), ab(49)]
    scrd = nc.dram_tensor("scr", [16, G, 49], f32, kind="Internal")

    with TileContext(nc) as tc:
        with tc.tile_pool(name="io", bufs=3) as io, \
             tc.tile_pool(name="mk", bufs=3) as mk, \
             tc.tile_pool(name="wk", bufs=2) as wk, \
             tc.tile_pool(name="ps", bufs=8, space="PSUM") as ps, \
             tc.tile_pool(name="big", bufs=1) as big, \
             tc.tile_pool(name="st", bufs=1) as st:
            consts_t = big.tile([P, 2 * Tn], f32)
            nc.sync.dma_start(out=consts_t[:], in_=constsd[:])
            nv_t = consts_t[:, 0:Tn]
            invn_t = consts_t[:, Tn:2 * Tn]

            # Gram results, evacuated per group: [112, 64*112] f32
            E_all = big.tile([W, G * W], f32)

            for g in range(G):
                xz_t = io.tile([P, CH * W + 16], bf16, tag="xz")
                nc.sync.dma_start(out=xz_t[:, 0:CH * W], in_=xzd[g])
                msk_t = mk.tile([P, 48], bf16, tag="mk")
                nc.sync.dma_start(out=msk_t[:], in_=mskd[g])

                # masked moving operand for chunks 3-5 (one DVE op, bf16 2x)
                rm = wk.tile([P, 3 * W], bf16, tag="rm")
                nc.vector.tensor_tensor(
                    out=rm[:].rearrange("p (c s k) -> p c s k", c=3, s=16),
                    in0=xz_t[:, 3 * W:6 * W].rearrange(
                        "p (c s k) -> p c s k", c=3, s=16),
                    in1=msk_t[:].rearrange("p (c s) -> p c s", c=3)
                        .unsqueeze(3).broadcast_to([P, 3, 16, 7]),
                    op=ALU.mult)

                psum_t = ps.tile([P, W], f32, tag="ps")
                for c in range(CH):
                    # lhsT padded to 128 cols (enables fast weight load);
                    # pad cols only write psum rows 112-127, never read.
                    lhs = xz_t[:, W * c:W * c + 128]
                    if c < 3:
                        rhs = xz_t[:, W * c:W * c + W]
                    else:
                        rhs = rm[:, W * (c - 3):W * (c - 3) + W]
                    nc.tensor.matmul(psum_t[:], lhsT=lhs, rhs=rhs,
                                     start=(c == 0), stop=(c == CH - 1))

                nc.scalar.activation(out=E_all[:, W * g:W * (g + 1)],
                                     in_=psum_t[0:W, :], func=AF.Copy)

            # gather per-sample 7x7 blocks sample-major, via DRAM:
            # hop B: 16 DMAs, E_all[7s+a, 112g + 7s+b] -> scr[s, g, 7a+b]
            for s in range(16):
                src = E_all[7 * s:7 * s + 7, :].rearrange(
                    "p (g c) -> p g c", g=G)[:, :, 7 * s:7 * s + 7]
                dst = scrd[s].rearrange("g (a b) -> a g b", a=7)
                nc.sync.dma_start(out=dst, in_=src)
            # hop C: scr[s, 8t+g8, ab] -> ep[16 g8 + s, 49 t + ab]
            ep = big.tile([P, Tn * 49], f32)
            nc.sync.dma_start(
                out=ep[:],
                in_=scrd[:].rearrange("s (t g8) ab -> g8 s t ab", t=Tn))

            # ---------------- epilogue (batched over [P, ..., Tn]) ----------
            ep_r = ep[:].rearrange("p (t a b) -> p a b t", t=Tn, a=7, b=7)
            mmv = ep_r[:, 0:3, 3:6, :]          # [P, i, j, Tn] = sum Xi*Yj
            sxv = ep_r[:, 0:3, 6, :]            # [P, i, Tn]
            syv = ep_r[:, 3:6, 6, :]            # [P, j, Tn]
            ep_v = ep[:].rearrange("p (t e) -> p t e", t=Tn)

            cnt = [0]

            def new(shape):
                cnt[0] += 1
                free = int(np.prod(shape[1:]))
                r = st.tile([P, free], f32, tag=f"e{cnt[0]}")
                ap = r[:]
                if len(shape) > 2:
                    names = " ".join(f"d{i}" for i in range(len(shape) - 1))
                    ap = ap.rearrange(f"p ({names}) -> p {names}",
                                      **{f"d{i}": int(shape[1 + i])
                                         for i in range(len(shape) - 1)})
                return ap

            def tt(a, b, op, shape=None):
                r = new(list(shape or a.shape))
                nc.vector.tensor_tensor(out=r, in0=a, in1=b, op=op)
                return r

            def ts(a, s1, op0, s2=None, op1=None):
                r = new(list(a.shape))
                if op1 is None:
                    nc.vector.tensor_scalar(out=r, in0=a, scalar1=s1,
                                            scalar2=None, op0=op0)
                else:
                    nc.vector.tensor_scalar(out=r, in0=a, scalar1=s1,
                                            scalar2=s2, op0=op0, op1=op1)
                return r

            def stt(a, s, b, op0, op1):
                r = new(list(a.shape))
                nc.vector.scalar_tensor_tensor(out=r, in0=a, scalar=s,
                                               in1=b, op0=op0, op1=op1)
                return r

            def act(a, func, scale=1.0, bias=0.0):
                r = new(list(a.shape))
                nc.scalar.activation(out=r, in_=a, func=func,
                                     scale=scale, bias=bias)
                return r

            def recip(a):
                r = new(list(a.shape))
                nc.vector.reciprocal(out=r, in_=a)
                return r

            def red_inner(a, n_keep):
                r = new([P, n_keep])
                nc.vector.tensor_reduce(out=r, in_=a,
                                        axis=mybir.AxisListType.X, op=ALU.add)
                return r

            def poly_eval(x, coeffs):
                g_ = ts(x, coeffs[0], ALU.mult)
                for c in coeffs[1:-1]:
                    g_ = stt(g_, c, x, ALU.add, ALU.mult)
                return ts(g_, coeffs[-1], ALU.add)

            # ssx = sum_i G[i,i] (e = 8i), ssy = sum_j G[3+j,3+j] (e = 24+8j)
            ssx = red_inner(ep_v[:, :, 0:17:8], Tn)
            ssy = red_inner(ep_v[:, :, 24:41:8], Tn)

            invn_b3 = invn_t.unsqueeze(1).broadcast_to([P, 3, Tn])

            # R_ij = m_ij - (sx_i * invn) * sy_j
            meanx = tt(sxv, invn_b3, ALU.mult)                       # [P,3,Tn]
            meanx_v = meanx.unsqueeze(2).broadcast_to([P, 3, 3, Tn])
            sy_v = syv.unsqueeze(1).broadcast_to([P, 3, 3, Tn])
            mxsy = tt(meanx_v, sy_v, ALU.mult)
            Rv = tt(mmv, mxsy, ALU.subtract, shape=[P, 3, 3, Tn])

            # e0 = ssx + ssy - (|sx|^2 + |sy|^2) * invn
            sx2 = tt(sxv, sxv, ALU.mult, shape=[P, 3, Tn])
            sy2 = tt(syv, syv, ALU.mult, shape=[P, 3, Tn])
            nrm = tt(sx2, sy2, ALU.add)
            nrms = red_inner(nrm.rearrange("p i t -> p t i"), Tn)
            ss = tt(ssx, ssy, ALU.add)
            nrmi = tt(nrms, invn_t, ALU.mult)
            e0 = tt(ss, nrmi, ALU.subtract)                          # [P,Tn]

            # A = R^T R (batched outer products over k)
            Av = new([P, 3, 3, Tn])
            for k in range(3):
                rk = Rv[:, k]
                rk_a = rk.unsqueeze(2).broadcast_to([P, 3, 3, Tn])
                rk_b = rk.unsqueeze(1).broadcast_to([P, 3, 3, Tn])
                if k == 0:
                    nc.vector.tensor_tensor(out=Av, in0=rk_a, in1=rk_b,
                                            op=ALU.mult)
                else:
                    pk = tt(rk_a, rk_b, ALU.mult)
                    nc.vector.tensor_tensor(out=Av, in0=Av, in1=pk, op=ALU.add)
            Aflat = Av.rearrange("p a b t -> p (a b) t")
            Adiag = Aflat[:, ::4]                                    # [P,3,Tn]

            q = ts(red_inner(Adiag.rearrange("p a t -> p t a"), Tn),
                   1.0 / 3.0, ALU.mult)                              # [P,Tn]
            q_b3 = q.unsqueeze(1).broadcast_to([P, 3, Tn])
            bdiag = tt(Adiag, q_b3, ALU.subtract)

            # p2 = sum(bdiag^2) + (sum(A^2) - sum(diag(A)^2))
            asq = tt(Aflat, Aflat, ALU.mult)
            allsq = red_inner(asq.rearrange("p a t -> p t a"), Tn)
            dsq = tt(Adiag, Adiag, ALU.mult)
            dsqs = red_inner(dsq.rearrange("p a t -> p t a"), Tn)
            bsq = tt(bdiag, bdiag, ALU.mult)
            bsqs = red_inner(bsq.rearrange("p a t -> p t a"), Tn)
            offs = tt(allsq, dsqs, ALU.subtract)
            p2 = tt(bsqs, offs, ALU.add)                             # [P,Tn]

            # log-space: p = (p2/6)^0.5 and invp^3 = (p2/6)^-1.5
            p2e = ts(p2, 1e-10, ALU.add)
            lnp2 = act(p2e, AF.Ln, scale=1.0 / 6.0)
            p_ = act(lnp2, AF.Exp, scale=0.5)
            ip3 = act(lnp2, AF.Exp, scale=-1.5)

            # batched determinants of W0=R and W1=B (= A - q I)
            Dw = new([P, 2, 3, 3, Tn])
            nc.vector.tensor_copy(Dw[:, 0], Rv)
            nc.vector.tensor_copy(Dw[:, 1], Av)
            Dw_diag = Dw.rearrange("p w a b t -> p w (a b) t")[:, 1, ::4]
            nc.vector.tensor_tensor(out=Dw_diag, in0=Adiag, in1=q_b3,
                                    op=ALU.subtract)

            def dsl(i, j):
                return Dw[:, :, i, j]                                # [P,2,Tn]

            u1 = tt(dsl(1, 1), dsl(2, 2), ALU.mult)
            u2 = tt(dsl(1, 2), dsl(2, 1), ALU.mult)
            cof0 = tt(dsl(0, 0), tt(u1, u2, ALU.subtract), ALU.mult)
            u3 = tt(dsl(1, 0), dsl(2, 2), ALU.mult)
            u4 = tt(dsl(1, 2), dsl(2, 0), ALU.mult)
            cof1 = tt(dsl(0, 1), tt(u3, u4, ALU.subtract), ALU.mult)
            u5 = tt(dsl(1, 0), dsl(2, 1), ALU.mult)
            u6 = tt(dsl(1, 1), dsl(2, 0), ALU.mult)
            cof2 = tt(dsl(0, 2), tt(u5, u6, ALU.subtract), ALU.mult)
            dets = tt(tt(cof0, cof1, ALU.subtract), cof2, ALU.add)   # [P,2,Tn]
            detR = dets[:, 0]
            detB = dets[:, 1]

            # r = clamp(0.5 * detB * invp^3, -1, 1)
            rr = tt(detB, ip3, ALU.mult, shape=[P, Tn])
            r_ = ts(rr, 0.5, ALU.mult, 1.0, ALU.min)
            r_ = ts(r_, -1.0, ALU.max)

            # acos(r)/3 via |r| polynomial (A&S 4.4.46) + reflection
            rneg = ts(r_, -1.0, ALU.mult)
            tabs = tt(r_, rneg, ALU.max)
            poly = poly_eval(tabs, [-0.0012624911, 0.0066700901, -0.0170881256,
                                    0.0308918810, -0.0501743046, 0.0889789874,
                                    -0.2145988016, 1.5707963050])
            u_ = ts(tabs, -1.0, ALU.mult, 1.0, ALU.add)
            u_ = ts(u_, 1e-30, ALU.add)
            sq1mt = act(act(u_, AF.Ln), AF.Exp, scale=0.5)
            acos_t = tt(poly, sq1mt, ALU.mult)
            ind = ts(r_, 0.0, ALU.is_ge)
            sgn = ts(ind, 2.0, ALU.mult, -1.0, ALU.add)
            pio = ts(ind, -math.pi, ALU.mult, math.pi, ALU.add)
            acos_r = tt(tt(acos_t, sgn, ALU.mult), pio, ALU.add)
            phi = ts(acos_r, 1.0 / 3.0, ALU.mult)

            # cos/sin Taylor on [0, pi/3]; cos(phi+2pi/3) = -.5 c - (v3/2) s
            z = tt(phi, phi, ALU.mult)
            cosp = poly_eval(z, [1.0 / 40320, -1.0 / 720, 1.0 / 24, -0.5, 1.0])
            sinp = poly_eval(z, [-1.0 / 5040, 1.0 / 120, -1.0 / 6, 1.0])
            sinp = tt(sinp, phi, ALU.mult)
            halfc = ts(cosp, -0.5, ALU.mult)
            cosp2 = stt(sinp, -math.sqrt(3.0) / 2.0, halfc, ALU.mult, ALU.add)

            twop = ts(p_, 2.0, ALU.mult)
            eigs = new([P, 3, Tn])
            e1t = tt(twop, cosp, ALU.mult)
            nc.vector.tensor_tensor(out=eigs[:, 0], in0=e1t, in1=q, op=ALU.add)
            e3t = tt(twop, cosp2, ALU.mult)
            nc.vector.tensor_tensor(out=eigs[:, 2], in0=e3t, in1=q, op=ALU.add)
            q3 = ts(q, 3.0, ALU.mult)
            e12 = tt(eigs[:, 0], eigs[:, 2], ALU.add)
            nc.vector.tensor_tensor(out=eigs[:, 1], in0=q3, in1=e12,
                                    op=ALU.subtract)

            eig_c = ts(eigs.rearrange("p k t -> p (k t)"), 0.0, ALU.max,
                       1e-30, ALU.add)                                # [P,3Tn]
            sv = act(act(eig_c, AF.Ln), AF.Exp, scale=0.5)
            sv = sv.rearrange("p (k t) -> p k t", k=3)

            dind = ts(detR, 0.0, ALU.is_ge)
            dsgn = ts(dind, 2.0, ALU.mult, -1.0, ALU.add)
            s12 = tt(sv[:, 0], sv[:, 1], ALU.add)
            ds3 = tt(dsgn, sv[:, 2], ALU.mult)
            trace = tt(s12, ds3, ALU.add)                             # [P,Tn]

            e_ = stt(trace, -2.0, e0, ALU.mult, ALU.add)
            e_ = ts(e_, 0.0, ALU.max)
            arg = tt(e_, invn_t, ALU.mult)
            arg = ts(arg, 1e-7, ALU.add)
            y0 = act(act(arg, AF.Ln), AF.Exp, scale=0.5)
            ry = recip(y0)
            ay = tt(arg, ry, ALU.mult)
            outv = ts(tt(y0, ay, ALU.add), 0.5, ALU.mult)

            nc.sync.dma_start(out=outd[:], in_=outv)

    nc.compile()

    # collapse redundant ACT table loads (all funcs used live in
    # natural_log_exp_and_others; retarget + drop dupes, keeping syncs)
    tables = list(get_activation_tables(nc.m.arch).keys())
    target = tables.index("natural_log_exp_and_others")
    for blk in nc.main_func.blocks:
        seen = False
        drop = []
        for inst in list(blk.instructions):
            if isinstance(inst, mybir.InstLoadActFuncSet):
                inst.act_func_set_id = target
                si = inst.sync_info
                has_sync = si is not None and (si.on_wait or si.on_update)
                if seen and not has_sync:
                    drop.append(inst)
                    continue
                seen = True
        for inst in drop:
            blk.instructions.remove(inst)
    return nc


def get_nc(n_tiles=T):
    if n_tiles not in _CACHE:
        _CACHE[n_tiles] = _build(n_tiles)
    return _CACHE[n_tiles]


def _prep_core_inputs(X, Y, nf, n_tiles):
    import ml_dtypes
    bf16 = ml_dtypes.bfloat16
    # xz[g, p, c*112 + 7s + k] = comp k of sample 16g+s, atom 128c+p
    V = np.empty((S, M, 7), np.float32)
    V[:, :, 0:3] = X.reshape(S, M, 3)
    V[:, :, 3:6] = Y.reshape(S, M, 3)
    V[:, :, 6] = 1.0
    xz = np.ascontiguousarray(
        V.reshape(G, 16, CH, P, 7).transpose(0, 3, 2, 1, 4)
    ).reshape(G, P, CH * W).astype(bf16)
    # msk[g, p, 16c + s] = (384 + 128c + p < n[16g+s])
    na = nf.reshape(G, 1, 1, 16)
    pa = np.arange(P, dtype=np.float32).reshape(1, P, 1, 1)
    ca = np.arange(3, dtype=np.float32).reshape(1, 1, 3, 1)
    msk = ((384.0 + 128.0 * ca + pa) < na).astype(bf16).reshape(G, P, 48)
    # consts: nv[p, t] = n[128t + p], invn likewise
    invn = (np.float32(1.0) / nf).astype(np.float32)
    consts = np.concatenate(
        [nf.reshape(T, P).T, invn.reshape(T, P).T], axis=1
    ).astype(np.float32)
    return {"xz": xz, "msk": msk, "consts": np.ascontiguousarray(consts)}


def kernel(input, target, num_atoms):
    from concourse.bass_utils import run_bass_kernel_spmd

    X = np.asarray(input, dtype=np.float32)
    Y = np.asarray(target, dtype=np.float32)
    nf = np.asarray(num_atoms).astype(np.float32)
    B = X.shape[0]
    assert B == NCORES * S, f"unexpected batch {B}"

    nc = get_nc(T)
    in_maps = []
    for c in range(NCORES):
        sl = slice(c * S, (c + 1) * S)
        in_maps.append(_prep_core_inputs(X[sl], Y[sl], nf[sl], T))
    res = run_bass_kernel_spmd(nc, in_maps, list(range(NCORES))).results
    out = np.empty((NCORES, S), np.float32)
    for c in range(NCORES):
        out[c] = res[c]["out"].T.reshape(S)   # out[p,t] -> sample 128t+p
    return out.reshape(B)


# revision 3
# speedup vs baseline: 1.9717x; 1.7193x over previous
"""Trainium2 Bass kernel for nn_Coords2RMSD (masked Kabsch RMSD loss).

Pure data parallel over 8 NeuronCores (1024 samples each). All 17
per-sample reductions (3x3 correlation, component sums, sums of squares)
are computed on the TensorEngine as a batched 7x7 Gram matrix: per group
of 16 samples, Z = [X1 X2 X3 Y1 Y2 Y3 one] columns (7 per sample, 112
cols) contract over atoms in 6 chunks of 128 partitions, accumulating
G = Z^T (mask * Z) in PSUM. Masking only the moving operand is exact
(the mask is idempotent); atoms < 384 are never masked (num_atoms >=
384), so only chunks 3-5 need the mask multiply (split DVE/GpSimd).
The whole input stream is DMA'd up-front into persistent SBUF tiles
(split across the sync and scalar HWDGE rings) so the TensorEngine
free-runs. Per-sample 7x7 diagonal blocks are gathered sample-major via
a through-DRAM DMA shuffle, then a closed-form 3x3 eigenvalue epilogue
(trig method) turns the reductions into the RMSD.
"""
import math
import numpy as np

P = 128          # partitions
M = 768          # max atoms
NCORES = 8
T = 8            # epilogue tiles (sample p of tile t is sample 128t+p)
S = 1024         # samples per core
G = 64           # sample groups per core (16 samples each)
W = 112          # matmul columns per group (16 samples x 7 comps)
GW = 688         # per-group stream width: 6*112 data + 16 zero pad
CH = 6           # contraction chunks of 128 atoms
NSUP = 8         # xz stream super-tiles (8 groups each)

_CACHE = {}


def _build(n_tiles):
    import concourse.bacc as bacc
    import concourse.mybir as mybir
    from concourse.tile import TileContext
    from concourse.hw_specs import get_activation_tables

    f32 = mybir.dt.float32
    bf16 = mybir.dt.bfloat16
    ALU = mybir.AluOpType
    AF = mybir.ActivationFunctionType

    Tn = T
    GS = G // NSUP               # groups per super-tile

    nc = bacc.Bacc()
    # xz: [sup][p][g-in-sup * 688 + c*112 + 7s + k]
    xzd = nc.declare_dram_parameter("xz", [NSUP, P, GS * GW], bf16,
                                    isOutput=False)
    mskd = nc.declare_dram_parameter("msk", [P, G * 48], bf16, isOutput=False)
    constsd = nc.declare_dram_parameter("consts", [P, 2 * Tn], f32,
                                        isOutput=False)
    outd = nc.declare_dram_parameter("out", [P, Tn], f32, isOutput=True)
    # scratch for the diagonal-block gather: [s(16), g(64), ab(49)]
    scrd = nc.dram_tensor("scr", [16, G, 49], f32, kind="Internal")

    with TileContext(nc) as tc:
        with tc.tile_pool(name="big", bufs=1) as big, \
             tc.tile_pool(name="wk", bufs=8) as wk, \
             tc.tile_pool(name="ps", bufs=8, space="PSUM") as ps, \
             tc.tile_pool(name="st", bufs=1) as st:
            consts_t = big.tile([P, 2 * Tn], f32)
            nc.sync.dma_start(out=consts_t[:], in_=constsd[:])
            nv_t = consts_t[:, 0:Tn]
            invn_t = consts_t[:, Tn:2 * Tn]

            msk_t = big.tile([P, G * 48], bf16)
            nc.scalar.dma_start(out=msk_t[:], in_=mskd[:])

            # whole xz stream resident in SBUF; alternate HWDGE rings
            xz_sup = []
            for j in range(NSUP):
                xt = big.tile([P, GS * GW], bf16, tag=f"xz{j}")
                eng = nc.sync if j % 2 == 0 else nc.scalar
                eng.dma_start(out=xt[:], in_=xzd[j])
                xz_sup.append(xt)

            # Gram results, evacuated per group: [112, 64*112] f32
            E_all = big.tile([W, G * W], f32)

            for g in range(G):
                xg = xz_sup[g // GS][:, (g % GS) * GW:(g % GS) * GW + GW]
                mg = msk_t[:, g * 48:(g + 1) * 48]

                # masked moving operand for chunks 3-5 (alternate DVE /
                # GpSimd so neither engine becomes the bottleneck)
                rm = wk.tile([P, 3 * W], bf16, tag="rm")
                meng = nc.vector if g % 2 == 0 else nc.gpsimd
                meng.tensor_tensor(
                    out=rm[:].rearrange("p (c s k) -> p c s k", c=3, s=16),
                    in0=xg[:, 3 * W:6 * W].rearrange(
                        "p (c s k) -> p c s k", c=3, s=16),
                    in1=mg.rearrange("p (c s) -> p c s", c=3)
                        .unsqueeze(3).broadcast_to([P, 3, 16, 7]),
                    op=ALU.mult)

                psum_t = ps.tile([P, W], f32, tag="ps")
                for c in range(CH):
                    # lhsT padded to 128 cols (pad cols only write psum
                    # rows 112-127, never read; host sends zeros there)
                    lhs = xg[:, W * c:W * c + 128]
                    if c < 3:
                        rhs = xg[:, W * c:W * c + W]
                    else:
                        rhs = rm[:, W * (c - 3):W * (c - 3) + W]
                    nc.tensor.matmul(psum_t[:], lhsT=lhs, rhs=rhs,
                                     start=(c == 0), stop=(c == CH - 1))

                nc.scalar.activation(out=E_all[:, W * g:W * (g + 1)],
                                     in_=psum_t[0:W, :], func=AF.Copy)

            # gather per-sample 7x7 blocks sample-major, via DRAM:
            # hop B: 16 DMAs, E_all[7s+a, 112g + 7s+b] -> scr[s, g, 7a+b]
            for s in range(16):
                src = E_all[7 * s:7 * s + 7, :].rearrange(
                    "p (g c) -> p g c", g=G)[:, :, 7 * s:7 * s + 7]
                dst = scrd[s].rearrange("g (a b) -> a g b", a=7)
                eng = nc.sync if s % 2 == 0 else nc.scalar
                eng.dma_start(out=dst, in_=src)
            # hop C: scr[s, 8t+g8, ab] -> ep[16 g8 + s, 49 t + ab]
            ep = big.tile([P, Tn * 49], f32)
            nc.sync.dma_start(
                out=ep[:],
                in_=scrd[:].rearrange("s (t g8) ab -> g8 s t ab", t=Tn))

            # ---------------- epilogue (batched over [P, ..., Tn]) ----------
            ep_r = ep[:].rearrange("p (t a b) -> p a b t", t=Tn, a=7, b=7)
            mmv = ep_r[:, 0:3, 3:6, :]          # [P, i, j, Tn] = sum Xi*Yj
            sxv = ep_r[:, 0:3, 6, :]            # [P, i, Tn]
            syv = ep_r[:, 3:6, 6, :]            # [P, j, Tn]
            ep_v = ep[:].rearrange("p (t e) -> p t e", t=Tn)

            cnt = [0]

            def new(shape):
                cnt[0] += 1
                free = int(np.prod(shape[1:]))
                r = st.tile([P, free], f32, tag=f"e{cnt[0]}")
                ap = r[:]
                if len(shape) > 2:
                    names = " ".join(f"d{i}" for i in range(len(shape) - 1))
                    ap = ap.rearrange(f"p ({names}) -> p {names}",
                                      **{f"d{i}": int(shape[1 + i])
                                         for i in range(len(shape) - 1)})
                return ap

            def tt(a, b, op, shape=None):
                r = new(list(shape or a.shape))
                nc.vector.tensor_tensor(out=r, in0=a, in1=b, op=op)
                return r

            def ts(a, s1, op0, s2=None, op1=None):
                r = new(list(a.shape))
                if op1 is None:
                    nc.vector.tensor_scalar(out=r, in0=a, scalar1=s1,
                                            scalar2=None, op0=op0)
                else:
                    nc.vector.tensor_scalar(out=r, in0=a, scalar1=s1,
                                            scalar2=s2, op0=op0, op1=op1)
                return r

            def stt(a, s, b, op0, op1):
                r = new(list(a.shape))
                nc.vector.scalar_tensor_tensor(out=r, in0=a, scalar=s,
                                               in1=b, op0=op0, op1=op1)
                return r

            def act(a, func, scale=1.0, bias=0.0):
                r = new(list(a.shape))
                nc.scalar.activation(out=r, in_=a, func=func,
                                     scale=scale, bias=bias)
                return r

            def recip(a):
                r = new(list(a.shape))
                nc.vector.reciprocal(out=r, in_=a)
                return r

            def red_inner(a, n_keep):
                r = new([P, n_keep])
                nc.vector.tensor_reduce(out=r, in_=a,
                                        axis=mybir.AxisListType.X, op=ALU.add)
                return r

            def poly_eval(x, coeffs):
                g_ = ts(x, coeffs[0], ALU.mult)
                for c in coeffs[1:-1]:
                    g_ = stt(g_, c, x, ALU.add, ALU.mult)
                return ts(g_, coeffs[-1], ALU.add)

            # ssx = sum_i G[i,i] (e = 8i), ssy = sum_j G[3+j,3+j] (e = 24+8j)
            ssx = red_inner(ep_v[:, :, 0:17:8], Tn)
            ssy = red_inner(ep_v[:, :, 24:41:8], Tn)

            invn_b3 = invn_t.unsqueeze(1).broadcast_to([P, 3, Tn])

            # R_ij = m_ij - (sx_i * invn) * sy_j
            meanx = tt(sxv, invn_b3, ALU.mult)                       # [P,3,Tn]
            meanx_v = meanx.unsqueeze(2).broadcast_to([P, 3, 3, Tn])
            sy_v = syv.unsqueeze(1).broadcast_to([P, 3, 3, Tn])
            mxsy = tt(meanx_v, sy_v, ALU.mult)
            Rv = tt(mmv, mxsy, ALU.subtract, shape=[P, 3, 3, Tn])

            # e0 = ssx + ssy - (|sx|^2 + |sy|^2) * invn
            sx2 = tt(sxv, sxv, ALU.mult, shape=[P, 3, Tn])
            sy2 = tt(syv, syv, ALU.mult, shape=[P, 3, Tn])
            nrm = tt(sx2, sy2, ALU.add)
            nrms = red_inner(nrm.rearrange("p i t -> p t i"), Tn)
            ss = tt(ssx, ssy, ALU.add)
            nrmi = tt(nrms, invn_t, ALU.mult)
            e0 = tt(ss, nrmi, ALU.subtract)                          # [P,Tn]

            # A = R^T R (batched outer products over k)
            Av = new([P, 3, 3, Tn])
            for k in range(3):
                rk = Rv[:, k]
                rk_a = rk.unsqueeze(2).broadcast_to([P, 3, 3, Tn])
                rk_b = rk.unsqueeze(1).broadcast_to([P, 3, 3, Tn])
                if k == 0:
                    nc.vector.tensor_tensor(out=Av, in0=rk_a, in1=rk_b,
                                            op=ALU.mult)
                else:
                    pk = tt(rk_a, rk_b, ALU.mult)
                    nc.vector.tensor_tensor(out=Av, in0=Av, in1=pk, op=ALU.add)
            Aflat = Av.rearrange("p a b t -> p (a b) t")
            Adiag = Aflat[:, ::4]                                    # [P,3,Tn]

            q = ts(red_inner(Adiag.rearrange("p a t -> p t a"), Tn),
                   1.0 / 3.0, ALU.mult)                              # [P,Tn]
            q_b3 = q.unsqueeze(1).broadcast_to([P, 3, Tn])
            bdiag = tt(Adiag, q_b3, ALU.subtract)

            # p2 = sum(bdiag^2) + (sum(A^2) - sum(diag(A)^2))
            asq = tt(Aflat, Aflat, ALU.mult)
            allsq = red_inner(asq.rearrange("p a t -> p t a"), Tn)
            dsq = tt(Adiag, Adiag, ALU.mult)
            dsqs = red_inner(dsq.rearrange("p a t -> p t a"), Tn)
            bsq = tt(bdiag, bdiag, ALU.mult)
            bsqs = red_inner(bsq.rearrange("p a t -> p t a"), Tn)
            offs = tt(allsq, dsqs, ALU.subtract)
            p2 = tt(bsqs, offs, ALU.add)                             # [P,Tn]

            # log-space: p = (p2/6)^0.5 and invp^3 = (p2/6)^-1.5
            p2e = ts(p2, 1e-10, ALU.add)
            lnp2 = act(p2e, AF.Ln, scale=1.0 / 6.0)
            p_ = act(lnp2, AF.Exp, scale=0.5)
            ip3 = act(lnp2, AF.Exp, scale=-1.5)

            # batched determinants of W0=R and W1=B (= A - q I)
            Dw = new([P, 2, 3, 3, Tn])
            nc.vector.tensor_copy(Dw[:, 0], Rv)
            nc.vector.tensor_copy(Dw[:, 1], Av)
            Dw_diag = Dw.rearrange("p w a b t -> p w (a b) t")[:, 1, ::4]
            nc.vector.tensor_tensor(out=Dw_diag, in0=Adiag, in1=q_b3,
                                    op=ALU.subtract)

            def dsl(i, j):
                return Dw[:, :, i, j]                                # [P,2,Tn]

            u1 = tt(dsl(1, 1), dsl(2, 2), ALU.mult)
            u2 = tt(dsl(1, 2), dsl(2, 1), ALU.mult)
            cof0 = tt(dsl(0, 0), tt(u1, u2, ALU.subtract), ALU.mult)
            u3 = tt(dsl(1, 0), dsl(2, 2), ALU.mult)
            u4 = tt(dsl(1, 2), dsl(2, 0), ALU.mult)
            cof1 = tt(dsl(0, 1), tt(u3, u4, ALU.subtract), ALU.mult)
            u5 = tt(dsl(1, 0), dsl(2, 1), ALU.mult)
            u6 = tt(dsl(1, 1), dsl(2, 0), ALU.mult)
            cof2 = tt(dsl(0, 2), tt(u5, u6, ALU.subtract), ALU.mult)
            dets = tt(tt(cof0, cof1, ALU.subtract), cof2, ALU.add)   # [P,2,Tn]
            detR = dets[:, 0]
            detB = dets[:, 1]

            # r = clamp(0.5 * detB * invp^3, -1, 1)
            rr = tt(detB, ip3, ALU.mult, shape=[P, Tn])
            r_ = ts(rr, 0.5, ALU.mult, 1.0, ALU.min)
            r_ = ts(r_, -1.0, ALU.max)

            # acos(r)/3 via |r| polynomial (A&S 4.4.46) + reflection
            rneg = ts(r_, -1.0, ALU.mult)
            tabs = tt(r_, rneg, ALU.max)
            poly = poly_eval(tabs, [-0.0012624911, 0.0066700901, -0.0170881256,
                                    0.0308918810, -0.0501743046, 0.0889789874,
                                    -0.2145988016, 1.5707963050])
            u_ = ts(tabs, -1.0, ALU.mult, 1.0, ALU.add)
            u_ = ts(u_, 1e-30, ALU.add)
            sq1mt = act(act(u_, AF.Ln), AF.Exp, scale=0.5)
            acos_t = tt(poly, sq1mt, ALU.mult)
            ind = ts(r_, 0.0, ALU.is_ge)
            sgn = ts(ind, 2.0, ALU.mult, -1.0, ALU.add)
            pio = ts(ind, -math.pi, ALU.mult, math.pi, ALU.add)
            acos_r = tt(tt(acos_t, sgn, ALU.mult), pio, ALU.add)
            phi = ts(acos_r, 1.0 / 3.0, ALU.mult)

            # cos/sin Taylor on [0, pi/3]; cos(phi+2pi/3) = -.5 c - (v3/2) s
            z = tt(phi, phi, ALU.mult)
            cosp = poly_eval(z, [1.0 / 40320, -1.0 / 720, 1.0 / 24, -0.5, 1.0])
            sinp = poly_eval(z, [-1.0 / 5040, 1.0 / 120, -1.0 / 6, 1.0])
            sinp = tt(sinp, phi, ALU.mult)
            halfc = ts(cosp, -0.5, ALU.mult)
            cosp2 = stt(sinp, -math.sqrt(3.0) / 2.0, halfc, ALU.mult, ALU.add)

            twop = ts(p_, 2.0, ALU.mult)
            eigs = new([P, 3, Tn])
            e1t = tt(twop, cosp, ALU.mult)
            nc.vector.tensor_tensor(out=eigs[:, 0], in0=e1t, in1=q, op=ALU.add)
            e3t = tt(twop, cosp2, ALU.mult)
            nc.vector.tensor_tensor(out=eigs[:, 2], in0=e3t, in1=q, op=ALU.add)
            q3 = ts(q, 3.0, ALU.mult)
            e12 = tt(eigs[:, 0], eigs[:, 2], ALU.add)
            nc.vector.tensor_tensor(out=eigs[:, 1], in0=q3, in1=e12,
                                    op=ALU.subtract)

            eig_c = ts(eigs.rearrange("p k t -> p (k t)"), 0.0, ALU.max,
                       1e-30, ALU.add)                                # [P,3Tn]
            sv = act(act(eig_c, AF.Ln), AF.Exp, scale=0.5)
            sv = sv.rearrange("p (k t) -> p k t", k=3)

            dind = ts(detR, 0.0, ALU.is_ge)
            dsgn = ts(dind, 2.0, ALU.mult, -1.0, ALU.add)
            s12 = tt(sv[:, 0], sv[:, 1], ALU.add)
            ds3 = tt(dsgn, sv[:, 2], ALU.mult)
            trace = tt(s12, ds3, ALU.add)                             # [P,Tn]

            e_ = stt(trace, -2.0, e0, ALU.mult, ALU.add)
            e_ = ts(e_, 0.0, ALU.max)
            arg = tt(e_, invn_t, ALU.mult)
            arg = ts(arg, 1e-7, ALU.add)
            y0 = act(act(arg, AF.Ln), AF.Exp, scale=0.5)
            ry = recip(y0)
            ay = tt(arg, ry, ALU.mult)
            outv = ts(tt(y0, ay, ALU.add), 0.5, ALU.mult)

            nc.sync.dma_start(out=outd[:], in_=outv)

    nc.compile()

    # collapse redundant ACT table loads (all funcs used live in
    # natural_log_exp_and_others; retarget + drop dupes, keeping syncs)
    tables = list(get_activation_tables(nc.m.arch).keys())
    target = tables.index("natural_log_exp_and_others")
    for blk in nc.main_func.blocks:
        seen = False
        drop = []
        for inst in list(blk.instructions):
            if isinstance(inst, mybir.InstLoadActFuncSet):
                inst.act_func_set_id = target
                si = inst.sync_info
                has_sync = si is not None and (si.on_wait or si.on_update)
                if seen and not has_sync:
                    drop.append(inst)
                    continue
                seen = True
        for inst in drop:
            blk.instructions.remove(inst)
    return nc


def get_nc(n_tiles=T):
    if n_tiles not in _CACHE:
        _CACHE[n_tiles] = _build(n_tiles)
    return _CACHE[n_tiles]


def _prep_core_inputs(X, Y, nf, n_tiles):
    import ml_dtypes
    bf16 = ml_dtypes.bfloat16
    GS = G // NSUP
    # xz[sup, p, gs*688 + c*112 + 7s + k] = comp k of sample 16g+s,
    # atom 128c+p (g = 8*sup + gs); cols 672:688 of each group zero
    V = np.zeros((S, M, 7), np.float32)
    V[:, :, 0:3] = X.reshape(S, M, 3)
    V[:, :, 3:6] = Y.reshape(S, M, 3)
    V[:, :, 6] = 1.0
    xz = np.zeros((G, P, GW), np.float32)
    xz[:, :, 0:CH * W] = V.reshape(G, 16, CH, P, 7).transpose(
        0, 3, 2, 1, 4).reshape(G, P, CH * W)
    xz = np.ascontiguousarray(
        xz.reshape(NSUP, GS, P, GW).transpose(0, 2, 1, 3)
    ).reshape(NSUP, P, GS * GW).astype(bf16)
    # msk[p, 48g + 16c + s] = (384 + 128c + p < n[16g+s])
    na = nf.reshape(G, 1, 1, 16)
    pa = np.arange(P, dtype=np.float32).reshape(1, P, 1, 1)
    ca = np.arange(3, dtype=np.float32).reshape(1, 1, 3, 1)
    msk = ((384.0 + 128.0 * ca + pa) < na).astype(bf16)   # [G, P, 3, 16]
    msk = np.ascontiguousarray(msk.transpose(1, 0, 2, 3)).reshape(P, G * 48)
    # consts: nv[p, t] = n[128t + p], invn likewise
    invn = (np.float32(1.0) / nf).astype(np.float32)
    consts = np.concatenate(
        [nf.reshape(T, P).T, invn.reshape(T, P).T], axis=1
    ).astype(np.float32)
    return {"xz": xz, "msk": msk, "consts": np.ascontiguousarray(consts)}


def kernel(input, target, num_atoms):
    from concourse.bass_utils import run_bass_kernel_spmd

    X = np.asarray(input, dtype=np.float32)
    Y = np.asarray(target, dtype=np.float32)
    nf = np.asarray(num_atoms).astype(np.float32)
    B = X.shape[0]
    assert B == NCORES * S, f"unexpected batch {B}"

    nc = get_nc(T)
    in_maps = []
    for c in range(NCORES):
        sl = slice(c * S, (c + 1) * S)
        in_maps.append(_prep_core_inputs(X[sl], Y[sl], nf[sl], T))
    res = run_bass_kernel_spmd(nc, in_maps, list(range(NCORES))).results
    out = np.empty((NCORES, S), np.float32)
    for c in range(NCORES):
        out[c] = res[c]["out"].T.reshape(S)   # out[p,t] -> sample 128t+p
    return out.reshape(B)


# revision 5
# speedup vs baseline: 2.5365x; 1.2865x over previous
"""Trainium2 Bass kernel for nn_Coords2RMSD (masked Kabsch RMSD loss).

Pure data parallel over 8 NeuronCores. Samples are globally sorted by
num_atoms and dealt round-robin to cores, so all cores share one
compiled schedule: per group of 16 samples, only ceil(max_n/128) atom
chunks are streamed/contracted, and at most the last chunk or two need
a per-sample mask. All 17 per-sample reductions (3x3 correlation,
component sums, sums of squares) are computed on the TensorEngine as a
batched 7x7 Gram matrix in fp8e4m3 (tolerance allows it; fp32 PSUM
accumulate): Z = [X1 X2 X3 Y1 Y2 Y3 one] columns, G = Z^T (mask*Z),
masking only the moving operand (the mask is idempotent). The whole
input stream is DMA'd up-front into persistent SBUF tiles across two
HWDGE rings so the TensorEngine free-runs. Per-sample 7x7 diagonal
blocks are gathered sample-major via a through-DRAM DMA shuffle, and a
closed-form 3x3 eigenvalue epilogue (trig method) turns the reductions
into the RMSD; extraction + epilogue run in two halves so the first
half hides under the main loop.
"""
import math
import numpy as np

P = 128          # partitions
M = 768          # max atoms
NCORES = 8
T = 8            # epilogue tiles (sample p of tile t is position 128t+p)
S = 1024         # samples per core
G = 64           # sample groups per core (16 samples each)
W = 112          # matmul columns per group (16 samples x 7 comps)
NSUP = 16        # xz stream load chunks (4 groups each)

_CACHE = {}


def _plan(na):
    """Global sort + deal; schedule shared by all cores."""
    na = np.asarray(na).astype(np.int64)
    order = np.argsort(na, kind="stable")
    n_pos = na[order].reshape(S, NCORES)      # [position, core]
    ngrp = n_pos.reshape(G, 16, NCORES)
    gmax = ngrp.max(axis=(1, 2))
    gmin = ngrp.min(axis=(1, 2))
    chunks = np.ceil(gmax / 128.0).astype(int)
    cmin = np.minimum(np.floor(gmin / 128.0).astype(int), chunks)
    nmask = chunks - cmin
    return order, tuple(int(c) for c in chunks), tuple(int(m) for m in nmask)


def _build(schedule):
    import concourse.bacc as bacc
    import concourse.mybir as mybir
    from concourse.tile import TileContext
    from concourse.hw_specs import get_activation_tables

    f32 = mybir.dt.float32
    fp8 = mybir.dt.float8e4
    ALU = mybir.AluOpType
    AF = mybir.ActivationFunctionType

    chunks, nmask = schedule
    Tn = T
    gw = [112 * c + 16 for c in chunks]          # stream width per group
    goff = np.concatenate([[0], np.cumsum(gw)]).astype(int)
    W_TOT = int(goff[-1])
    moff = np.concatenate([[0], np.cumsum([16 * m for m in nmask])]).astype(int)
    MK_TOT = max(int(moff[-1]), 16)

    nc = bacc.Bacc()
    xzd = nc.declare_dram_parameter("xz", [P, W_TOT], fp8, isOutput=False)
    mskd = nc.declare_dram_parameter("msk", [P, MK_TOT], fp8, isOutput=False)
    constsd = nc.declare_dram_parameter("consts", [P, 2 * Tn], f32,
                                        isOutput=False)
    outd = nc.declare_dram_parameter("out", [P, Tn], f32, isOutput=True)
    # per-half scratch for the diagonal-block gather: [s(16), g(32), ab(49)]
    scrd = [nc.dram_tensor(f"scr{h}", [16, G // 2, 49], f32, kind="Internal")
            for h in range(2)]

    with TileContext(nc) as tc:
        with tc.tile_pool(name="big", bufs=1) as big, \
             tc.tile_pool(name="wk", bufs=8) as wk, \
             tc.tile_pool(name="ps", bufs=8, space="PSUM") as ps, \
             tc.tile_pool(name="st", bufs=1) as st:
            consts_t = big.tile([P, 2 * Tn], f32)
            nc.sync.dma_start(out=consts_t[:], in_=constsd[:])
            invn_t = consts_t[:, Tn:2 * Tn]

            msk_t = big.tile([P, MK_TOT], fp8)
            nc.scalar.dma_start(out=msk_t[:], in_=mskd[:])

            # whole xz stream resident in SBUF; alternate HWDGE rings
            xz_sup = []
            sup_bounds = []
            gps = G // NSUP
            for j in range(NSUP):
                lo, hi = int(goff[4 * j]), int(goff[4 * (j + 1)])
                xt = big.tile([P, hi - lo], fp8, tag=f"xz{j}")
                eng = nc.sync if j % 2 == 0 else nc.scalar
                eng.dma_start(out=xt[:], in_=xzd[:, lo:hi])
                xz_sup.append(xt)
                sup_bounds.append(lo)

            # Gram results per half: [112, 32*112] f32
            E_h = [big.tile([W, (G // 2) * W], f32, tag=f"ea{h}", name=f"ea{h}")
                   for h in range(2)]

            for g in range(G):
                ch, nm = chunks[g], nmask[g]
                cmin = ch - nm
                j = g // gps
                base = int(goff[g]) - sup_bounds[j]
                xg = xz_sup[j][:, base:base + gw[g]]

                rm = None
                if nm > 0:
                    rm = wk.tile([P, 112 * nm], fp8, tag="rm")
                    mg = msk_t[:, int(moff[g]):int(moff[g]) + 16 * nm]
                    nc.vector.tensor_tensor(
                        out=rm[:].rearrange("p (c s k) -> p c s k", c=nm, s=16),
                        in0=xg[:, 112 * cmin:112 * ch].rearrange(
                            "p (c s k) -> p c s k", c=nm, s=16),
                        in1=mg.rearrange("p (c s) -> p c s", c=nm)
                            .unsqueeze(3).broadcast_to([P, nm, 16, 7]),
                        op=ALU.mult)

                psum_t = ps.tile([P, W], f32, tag="ps")
                for c in range(ch):
                    # lhsT padded to 128 cols (pad cols only write psum
                    # rows 112-127, never read; host zeros the tail pad)
                    lhs = xg[:, W * c:W * c + 128]
                    if c < cmin:
                        rhs = xg[:, W * c:W * c + W]
                    else:
                        rhs = rm[:, W * (c - cmin):W * (c - cmin) + W]
                    nc.tensor.matmul(psum_t[:], lhsT=lhs, rhs=rhs,
                                     start=(c == 0), stop=(c == ch - 1))

                gh = g % (G // 2)
                nc.scalar.activation(out=E_h[g // (G // 2)][:, W * gh:W * (gh + 1)],
                                     in_=psum_t[0:W, :], func=AF.Copy)

            # ---------------- extraction + epilogue, per half ----------
            cnt = [0]

            def new(shape, nfree=None):
                cnt[0] += 1
                free = int(np.prod(shape[1:]))
                r = st.tile([P, free], f32, tag=f"e{cnt[0]}")
                ap = r[:]
                if len(shape) > 2:
                    names = " ".join(f"d{i}" for i in range(len(shape) - 1))
                    ap = ap.rearrange(f"p ({names}) -> p {names}",
                                      **{f"d{i}": int(shape[1 + i])
                                         for i in range(len(shape) - 1)})
                return ap

            def tt(a, b, op, shape=None):
                r = new(list(shape or a.shape))
                nc.vector.tensor_tensor(out=r, in0=a, in1=b, op=op)
                return r

            def ts(a, s1, op0, s2=None, op1=None):
                r = new(list(a.shape))
                if op1 is None:
                    nc.vector.tensor_scalar(out=r, in0=a, scalar1=s1,
                                            scalar2=None, op0=op0)
                else:
                    nc.vector.tensor_scalar(out=r, in0=a, scalar1=s1,
                                            scalar2=s2, op0=op0, op1=op1)
                return r

            def stt(a, s, b, op0, op1):
                r = new(list(a.shape))
                nc.vector.scalar_tensor_tensor(out=r, in0=a, scalar=s,
                                               in1=b, op0=op0, op1=op1)
                return r

            def act(a, func, scale=1.0, bias=0.0):
                r = new(list(a.shape))
                nc.scalar.activation(out=r, in_=a, func=func,
                                     scale=scale, bias=bias)
                return r

            def recip(a):
                r = new(list(a.shape))
                nc.vector.reciprocal(out=r, in_=a)
                return r

            def red_inner(a, n_keep):
                r = new([P, n_keep])
                nc.vector.tensor_reduce(out=r, in_=a,
                                        axis=mybir.AxisListType.X, op=ALU.add)
                return r

            def poly_eval(x, coeffs):
                g_ = ts(x, coeffs[0], ALU.mult)
                for c in coeffs[1:-1]:
                    g_ = stt(g_, c, x, ALU.add, ALU.mult)
                return ts(g_, coeffs[-1], ALU.add)

            hopb_engs = [nc.sync, nc.scalar, nc.gpsimd]

            for h in range(2):
                Th = Tn // 2
                # hop B: E_h[7s+a, 112g + 7s+b] -> scr[s, g, 7a+b]
                for s in range(16):
                    src = E_h[h][7 * s:7 * s + 7, :].rearrange(
                        "p (g c) -> p g c", g=G // 2)[:, :, 7 * s:7 * s + 7]
                    dst = scrd[h][s].rearrange("g (a b) -> a g b", a=7)
                    hopb_engs[s % 3].dma_start(out=dst, in_=src)
                # hop C: scr[s, 8t+g8, ab] -> ep[16 g8 + s, 49 t + ab]
                ep = big.tile([P, Th * 49], f32, tag=f"ep{h}")
                nc.sync.dma_start(
                    out=ep[:],
                    in_=scrd[h][:].rearrange("s (t g8) ab -> g8 s t ab", t=Th))

                ep_r = ep[:].rearrange("p (t a b) -> p a b t", t=Th, a=7, b=7)
                mmv = ep_r[:, 0:3, 3:6, :]      # [P, i, j, Th] = sum Xi*Yj
                sxv = ep_r[:, 0:3, 6, :]
                syv = ep_r[:, 3:6, 6, :]
                ep_v = ep[:].rearrange("p (t e) -> p t e", t=Th)
                invn_h = invn_t[:, Th * h:Th * (h + 1)]

                # ssx = sum_i G[i,i], ssy = sum_j G[3+j,3+j]
                ssx = red_inner(ep_v[:, :, 0:17:8], Th)
                ssy = red_inner(ep_v[:, :, 24:41:8], Th)

                invn_b3 = invn_h.unsqueeze(1).broadcast_to([P, 3, Th])

                # R_ij = m_ij - (sx_i * invn) * sy_j
                meanx = tt(sxv, invn_b3, ALU.mult)
                meanx_v = meanx.unsqueeze(2).broadcast_to([P, 3, 3, Th])
                sy_v = syv.unsqueeze(1).broadcast_to([P, 3, 3, Th])
                mxsy = tt(meanx_v, sy_v, ALU.mult)
                Rv = tt(mmv, mxsy, ALU.subtract, shape=[P, 3, 3, Th])

                # e0 = ssx + ssy - (|sx|^2 + |sy|^2) * invn
                sx2 = tt(sxv, sxv, ALU.mult, shape=[P, 3, Th])
                sy2 = tt(syv, syv, ALU.mult, shape=[P, 3, Th])
                nrm = tt(sx2, sy2, ALU.add)
                nrms = red_inner(nrm.rearrange("p i t -> p t i"), Th)
                ss = tt(ssx, ssy, ALU.add)
                nrmi = tt(nrms, invn_h, ALU.mult)
                e0 = tt(ss, nrmi, ALU.subtract)                      # [P,Th]

                # A = R^T R (batched outer products over k)
                Av = new([P, 3, 3, Th])
                for k in range(3):
                    rk = Rv[:, k]
                    rk_a = rk.unsqueeze(2).broadcast_to([P, 3, 3, Th])
                    rk_b = rk.unsqueeze(1).broadcast_to([P, 3, 3, Th])
                    if k == 0:
                        nc.vector.tensor_tensor(out=Av, in0=rk_a, in1=rk_b,
                                                op=ALU.mult)
                    else:
                        pk = tt(rk_a, rk_b, ALU.mult)
                        nc.vector.tensor_tensor(out=Av, in0=Av, in1=pk,
                                                op=ALU.add)
                Aflat = Av.rearrange("p a b t -> p (a b) t")
                Adiag = Aflat[:, ::4]                                # [P,3,Th]

                q = ts(red_inner(Adiag.rearrange("p a t -> p t a"), Th),
                       1.0 / 3.0, ALU.mult)                          # [P,Th]
                q_b3 = q.unsqueeze(1).broadcast_to([P, 3, Th])
                bdiag = tt(Adiag, q_b3, ALU.subtract)

                # p2 = sum(bdiag^2) + (sum(A^2) - sum(diag(A)^2))
                asq = tt(Aflat, Aflat, ALU.mult)
                allsq = red_inner(asq.rearrange("p a t -> p t a"), Th)
                dsq = tt(Adiag, Adiag, ALU.mult)
                dsqs = red_inner(dsq.rearrange("p a t -> p t a"), Th)
                bsq = tt(bdiag, bdiag, ALU.mult)
                bsqs = red_inner(bsq.rearrange("p a t -> p t a"), Th)
                offs = tt(allsq, dsqs, ALU.subtract)
                p2 = tt(bsqs, offs, ALU.add)                         # [P,Th]

                # log-space: p = (p2/6)^0.5 and invp^3 = (p2/6)^-1.5
                p2e = ts(p2, 1e-10, ALU.add)
                lnp2 = act(p2e, AF.Ln, scale=1.0 / 6.0)
                p_ = act(lnp2, AF.Exp, scale=0.5)
                ip3 = act(lnp2, AF.Exp, scale=-1.5)

                # batched determinants of W0=R and W1=B (= A - q I)
                Dw = new([P, 2, 3, 3, Th])
                nc.vector.tensor_copy(Dw[:, 0], Rv)
                nc.vector.tensor_copy(Dw[:, 1], Av)
                Dw_diag = Dw.rearrange("p w a b t -> p w (a b) t")[:, 1, ::4]
                nc.vector.tensor_tensor(out=Dw_diag, in0=Adiag, in1=q_b3,
                                        op=ALU.subtract)

                def dsl(i, j):
                    return Dw[:, :, i, j]                            # [P,2,Th]

                u1 = tt(dsl(1, 1), dsl(2, 2), ALU.mult)
                u2 = tt(dsl(1, 2), dsl(2, 1), ALU.mult)
                cof0 = tt(dsl(0, 0), tt(u1, u2, ALU.subtract), ALU.mult)
                u3 = tt(dsl(1, 0), dsl(2, 2), ALU.mult)
                u4 = tt(dsl(1, 2), dsl(2, 0), ALU.mult)
                cof1 = tt(dsl(0, 1), tt(u3, u4, ALU.subtract), ALU.mult)
                u5 = tt(dsl(1, 0), dsl(2, 1), ALU.mult)
                u6 = tt(dsl(1, 1), dsl(2, 0), ALU.mult)
                cof2 = tt(dsl(0, 2), tt(u5, u6, ALU.subtract), ALU.mult)
                dets = tt(tt(cof0, cof1, ALU.subtract), cof2, ALU.add)
                detR = dets[:, 0]
                detB = dets[:, 1]

                # r = clamp(0.5 * detB * invp^3, -1, 1)
                rr = tt(detB, ip3, ALU.mult, shape=[P, Th])
                r_ = ts(rr, 0.5, ALU.mult, 1.0, ALU.min)
                r_ = ts(r_, -1.0, ALU.max)

                # acos(r) = pi/2 + sign(r) * (poly(|r|)*sqrt(1-|r|) - pi/2)
                tabs = act(r_, AF.Abs)
                poly = poly_eval(tabs, [-0.0012624911, 0.0066700901,
                                        -0.0170881256, 0.0308918810,
                                        -0.0501743046, 0.0889789874,
                                        -0.2145988016, 1.5707963050])
                u_ = ts(tabs, -1.0, ALU.mult, 1.0, ALU.add)
                sq1mt = act(act(u_, AF.Ln), AF.Exp, scale=0.5)
                sgn = act(r_, AF.Sign)
                pq = tt(poly, sq1mt, ALU.mult)
                inner = ts(pq, -math.pi / 2.0, ALU.add)
                sm = tt(sgn, inner, ALU.mult)
                phi = ts(sm, 1.0 / 3.0, ALU.mult, math.pi / 6.0, ALU.add)

                # cos/sin Taylor on [0,pi/3]; cos(phi+2pi/3) = -.5c - (v3/2)s
                z = tt(phi, phi, ALU.mult)
                cosp = poly_eval(z, [1.0 / 40320, -1.0 / 720, 1.0 / 24,
                                     -0.5, 1.0])
                sinp = poly_eval(z, [-1.0 / 5040, 1.0 / 120, -1.0 / 6, 1.0])
                sinp = tt(sinp, phi, ALU.mult)
                halfc = ts(cosp, -0.5, ALU.mult)
                cosp2 = stt(sinp, -math.sqrt(3.0) / 2.0, halfc,
                            ALU.mult, ALU.add)

                twop = ts(p_, 2.0, ALU.mult)
                eigs = new([P, 3, Th])
                e1t = tt(twop, cosp, ALU.mult)
                nc.vector.tensor_tensor(out=eigs[:, 0], in0=e1t, in1=q,
                                        op=ALU.add)
                e3t = tt(twop, cosp2, ALU.mult)
                nc.vector.tensor_tensor(out=eigs[:, 2], in0=e3t, in1=q,
                                        op=ALU.add)
                q3 = ts(q, 3.0, ALU.mult)
                e12 = tt(eigs[:, 0], eigs[:, 2], ALU.add)
                nc.vector.tensor_tensor(out=eigs[:, 1], in0=q3, in1=e12,
                                        op=ALU.subtract)

                eig_c = ts(eigs.rearrange("p k t -> p (k t)"), 0.0, ALU.max,
                           1e-30, ALU.add)                           # [P,3Th]
                sv = act(act(eig_c, AF.Ln), AF.Exp, scale=0.5)
                sv = sv.rearrange("p (k t) -> p k t", k=3)

                dsgn = act(detR, AF.Sign)
                s12 = tt(sv[:, 0], sv[:, 1], ALU.add)
                ds3 = tt(dsgn, sv[:, 2], ALU.mult)
                trace = tt(s12, ds3, ALU.add)                        # [P,Th]

                e_ = stt(trace, -2.0, e0, ALU.mult, ALU.add)
                e_ = ts(e_, 0.0, ALU.max)
                arg = tt(e_, invn_h, ALU.mult)
                arg = ts(arg, 1e-7, ALU.add)
                y0 = act(act(arg, AF.Ln), AF.Exp, scale=0.5)
                ry = recip(y0)
                ay = tt(arg, ry, ALU.mult)
                outv = ts(tt(y0, ay, ALU.add), 0.5, ALU.mult)

                nc.sync.dma_start(out=outd[:, Th * h:Th * (h + 1)], in_=outv)

    nc.compile()

    # collapse redundant ACT table loads (all funcs used live in
    # natural_log_exp_and_others; retarget + drop dupes, keeping syncs)
    tables = list(get_activation_tables(nc.m.arch).keys())
    target = tables.index("natural_log_exp_and_others")
    for blk in nc.main_func.blocks:
        seen = False
        drop = []
        for inst in list(blk.instructions):
            if isinstance(inst, mybir.InstLoadActFuncSet):
                inst.act_func_set_id = target
                si = inst.sync_info
                has_sync = si is not None and (si.on_wait or si.on_update)
                if seen and not has_sync:
                    drop.append(inst)
                    continue
                seen = True
        for inst in drop:
            blk.instructions.remove(inst)
    return nc


def get_nc_for(num_atoms):
    _, chunks, nmask = _plan(num_atoms)
    key = (chunks, nmask)
    if key not in _CACHE:
        _CACHE[key] = _build(key)
    return _CACHE[key]


def _prep_all(X, Y, nf):
    """Host prep: sort+deal, pack per-core fp8 streams."""
    import ml_dtypes
    fp8 = ml_dtypes.float8_e4m3
    na = np.asarray(nf).astype(np.int64)
    order, chunks, nmask = _plan(na)
    gw = [112 * c + 16 for c in chunks]
    goff = np.concatenate([[0], np.cumsum(gw)]).astype(int)
    W_TOT = int(goff[-1])
    moff = np.concatenate([[0], np.cumsum([16 * m for m in nmask])]).astype(int)
    MK_TOT = max(int(moff[-1]), 16)

    cols = order.reshape(S, NCORES)           # [position, core] -> orig idx
    in_maps = []
    for c in range(NCORES):
        idx = cols[:, c]
        n_c = na[idx].astype(np.float32)
        V = np.zeros((S, M, 7), np.float32)
        V[:, :, 0:3] = X[idx].reshape(S, M, 3)
        V[:, :, 3:6] = Y[idx].reshape(S, M, 3)
        V[:, :, 6] = 1.0
        xz = np.zeros((P, W_TOT), np.float32)
        msk = np.zeros((P, MK_TOT), np.float32)
        pa = np.arange(P, dtype=np.float32)
        for g in range(G):
            ch, nm = chunks[g], nmask[g]
            blk = V[16 * g:16 * g + 16, 0:128 * ch].reshape(
                16, ch, P, 7).transpose(2, 1, 0, 3).reshape(P, ch * W)
            xz[:, goff[g]:goff[g] + ch * W] = blk
            if nm:
                cs = np.arange(ch - nm, ch, dtype=np.float32)
                mg = ((128.0 * cs[None, :, None] + pa[:, None, None])
                      < n_c[16 * g:16 * g + 16][None, None, :])
                msk[:, moff[g]:moff[g + 1]] = mg.reshape(P, 16 * nm)
        invn = (np.float32(1.0) / n_c).astype(np.float32)
        consts = np.concatenate(
            [n_c.reshape(T, P).T, invn.reshape(T, P).T], axis=1
        ).astype(np.float32)
        in_maps.append({"xz": xz.astype(fp8), "msk": msk.astype(fp8),
                        "consts": np.ascontiguousarray(consts)})
    return in_maps, order


def kernel(input, target, num_atoms):
    from concourse.bass_utils import run_bass_kernel_spmd

    X = np.asarray(input, dtype=np.float32)
    Y = np.asarray(target, dtype=np.float32)
    B = X.shape[0]
    assert B == NCORES * S, f"unexpected batch {B}"

    nc = get_nc_for(num_atoms)
    in_maps, order = _prep_all(X, Y, num_atoms)
    res = run_bass_kernel_spmd(nc, in_maps, list(range(NCORES))).results
    out = np.empty(B, np.float32)
    cols = order.reshape(S, NCORES)
    for c in range(NCORES):
        out[cols[:, c]] = res[c]["out"].T.reshape(S)  # [p,t] -> pos 128t+p
    return out
